# revision 18
# baseline (speedup 1.0000x reference)
"""Trainium2 Bass kernel for nn_BasicLSTM (single-step LSTM cell variant).

Reference computation (B=4096, D=1024, S=1024):
    pre_f = inputs @ w_f + h0 @ u_f + b_f
    f     = sigmoid(pre_f)
    i     = sigmoid(inputs @ w_i + h0 @ u_i + b_i)
    o     = sigmoid(inputs @ w_o + h0 @ u_o + b_o)
    c_new = f * c0 + f * i          (input_cell reuses the forget gate)
    h_new = o * tanh(c_new)
    returns (h_new, c_new)

Sharding: batch 4-way x state 2-way over 8 NeuronCores. Core c handles
batch rows [ (c//2)*1024 : (c//2+1)*1024 ) and state cols
[ (c%2)*512 : (c%2+1)*512 ). Host-side prep per core:
    xt  = concat([inputs_rows, h0_rows], 1).T           # [2048, 1024]
    w_g = concat([w_g[:, cols], u_g[:, cols]], 0)       # [2048, 512]
so the device kernel is three plain matmuls (K=2048 contraction on the
partition axis) plus fused elementwise, no on-device transposes.
Matmuls run as float32r (full PE column rate at N=512).

Structure: phase f is k-major across all 8 PSUM banks so the PE
consumes each (xt_k, wf_k) DMA pair as it lands (phase f runs at
HBM-bandwidth parity); phases i and o are batch-tile-major with fused
bias-add / sigmoid / elementwise / stores. All DMAs issue from the
sync engine in exact consumption order.
"""

import sys

sys.path.insert(0, "/opt/trn_rl_repo")
import ml_dtypes

import numpy as np

B, D, S = 4096, 1024, 1024
N_CORES = 8
BB, SB = 4, 2          # batch blocks x state blocks
B_CORE = B // BB       # 1024 rows per core
S_CORE = S // SB       # 512 state cols per core
K = D + S              # 2048 contraction
KT = K // 128          # 16 k-tiles
BT = B_CORE // 128     # 8 batch tiles per core

_CACHE: dict = {}


def _build_nc():
    import concourse.mybir as mybir
    import concourse.tile as tile
    from concourse import bacc

    f32 = mybir.dt.float32
    f32r = mybir.dt.float16  # fp16 variant

    nc = bacc.Bacc("TRN2", target_bir_lowering=False, debug=False,
                   num_devices=N_CORES)

    xt = nc.dram_tensor("xt", [K, B_CORE], f32r, kind="ExternalInput")
    w = {g: nc.dram_tensor(f"w{g}", [K, S_CORE], f32r, kind="ExternalInput")
         for g in "fio"}
    b = {g: nc.dram_tensor(f"b{g}", [128, S_CORE], f32,
                           kind="ExternalInput") for g in "fio"}
    c0 = nc.dram_tensor("c0", [B_CORE, S_CORE], f32, kind="ExternalInput")
    ho = nc.dram_tensor("ho", [B_CORE, S_CORE], f32, kind="ExternalOutput")
    co = nc.dram_tensor("co", [B_CORE, S_CORE], f32, kind="ExternalOutput")

    xt_r = xt.ap().rearrange("(kt p) n -> kt p n", p=128)
    w_r = {g: w[g].ap().rearrange("(kt p) n -> kt p n", p=128) for g in "fio"}
    c0_r = c0.ap().rearrange("(bt p) n -> bt p n", p=128)
    ho_r = ho.ap().rearrange("(bt p) n -> bt p n", p=128)
    co_r = co.ap().rearrange("(bt p) n -> bt p n", p=128)

    SIG = mybir.ActivationFunctionType.Sigmoid
    TANH = mybir.ActivationFunctionType.Tanh

    with tile.TileContext(nc) as tc:
        with (
            tc.tile_pool(name="xtp", bufs=KT) as xtp,
            tc.tile_pool(name="wp", bufs=2 * KT) as wp,
            tc.tile_pool(name="biasp", bufs=1) as biasp,
            tc.tile_pool(name="c0p", bufs=BT) as c0p,
            tc.tile_pool(name="sigfp", bufs=BT) as sigfp,
            tc.tile_pool(name="t1p", bufs=BT) as t1p,
            tc.tile_pool(name="workp", bufs=2) as workp,
            tc.tile_pool(name="psp", bufs=8, space="PSUM") as psp,
        ):
            w_tiles = {g: [None] * KT for g in "fio"}
            xt_tiles = []
            bias_t = {}

            # ---- DMA issue in exact consumption order (all on sync) ----
            # phase f stream: (xt_k, wf_k) pairs; bias_f early (tiny)
            for k in range(KT):
                xtt = xtp.tile([128, B_CORE], f32r, name=f"xt_{k}", tag="xt")
                if k < 2:
                    # split the first loads so MM #0 starts sooner
                    h = B_CORE // 2
                    nc.sync.dma_start(out=xtt[:, :h], in_=xt_r[k][:, :h])
                    nc.sync.dma_start(out=xtt[:, h:], in_=xt_r[k][:, h:])
                else:
                    nc.sync.dma_start(out=xtt[:], in_=xt_r[k])
                xt_tiles.append(xtt)
                wt = wp.tile([128, S_CORE], f32r, name=f"wf_{k}", tag="w")
                nc.sync.dma_start(out=wt[:], in_=w_r["f"][k])
                w_tiles["f"][k] = wt
                if k == 0:
                    bt_ = biasp.tile([128, S_CORE], f32, name="bias_f",
                                     tag="bias_f")
                    nc.sync.dma_start(out=bt_[:], in_=b["f"].ap())
                    bias_t["f"] = bt_
            for k in range(KT):
                wt = wp.tile([128, S_CORE], f32r, name=f"wi_{k}", tag="w")
                nc.sync.dma_start(out=wt[:], in_=w_r["i"][k])
                w_tiles["i"][k] = wt
            for g in "io":
                bt_ = biasp.tile([128, S_CORE], f32, name=f"bias_{g}",
                                 tag=f"bias_{g}")
                nc.sync.dma_start(out=bt_[:], in_=b[g].ap())
                bias_t[g] = bt_
            c0_tiles = []
            for bt in range(BT):
                c0t = c0p.tile([128, S_CORE], f32, name=f"c0_{bt}", tag="c0")
                nc.sync.dma_start(out=c0t[:], in_=c0_r[bt])
                c0_tiles.append(c0t)
            for k in range(KT):
                wt = wp.tile([128, S_CORE], f32r, name=f"wo_{k}", tag="w")
                nc.sync.dma_start(out=wt[:], in_=w_r["o"][k])
                w_tiles["o"][k] = wt

            # ---- phase f: k-major over all 8 PSUM banks ----
            ps_f = [psp.tile([128, S_CORE], f32, name=f"psf_{bt}", tag="ps")
                    for bt in range(BT)]
            for k in range(KT):
                for bt in range(BT):
                    nc.tensor.matmul(
                        ps_f[bt][:],
                        xt_tiles[k][:, bt * 128:(bt + 1) * 128],
                        w_tiles["f"][k][:],
                        start=(k == 0), stop=(k == KT - 1))
            sig_f = []
            for bt in range(BT):
                nc.vector.tensor_add(ps_f[bt][:], ps_f[bt][:],
                                     bias_t["f"][:])
                sf = sigfp.tile([128, S_CORE], f32, name=f"sigf_{bt}",
                                tag="sigf")
                nc.scalar.activation(sf[:], ps_f[bt][:], SIG)
                sig_f.append(sf)

            def gate_psum(g, bt, bias=True):
                ps = psp.tile([128, S_CORE], f32, name=f"ps_{g}_{bt}",
                              tag="ps")
                for k in range(KT):
                    nc.tensor.matmul(
                        ps[:],
                        xt_tiles[k][:, bt * 128:(bt + 1) * 128],
                        w_tiles[g][k][:],
                        start=(k == 0), stop=(k == KT - 1))
                if bias:
                    nc.vector.tensor_add(ps[:], ps[:], bias_t[g][:])
                return ps

            # ---- phase i: stage t1 = c0 + sigmoid(pre_i), btile-major ----
            t1s = []
            for bt in range(BT):
                ps = gate_psum("i", bt)
                t1 = t1p.tile([128, S_CORE], f32, name=f"t1_{bt}", tag="t1")
                nc.scalar.activation(t1[:], ps[:], SIG)
                nc.vector.tensor_add(t1[:], t1[:], c0_tiles[bt][:])
                t1s.append(t1)

            # ---- phase o: finish per batch tile ----
            # elementwise runs as two half-width pipelined chains so the
            # post-matmul latency tail is halved
            HS = S_CORE // 2
            for bt in range(BT):
                ps = gate_psum("o", bt, bias=False)
                so = workp.tile([128, S_CORE], f32, name=f"sigo_{bt}",
                                tag="sigo")
                cn = workp.tile([128, S_CORE], f32, name=f"cnew_{bt}",
                                tag="cnew")
                hn = workp.tile([128, S_CORE], f32, name=f"hnew_{bt}",
                                tag="hnew")
                th = sig_f[bt]  # tanh reuses the sig_f slot
                for h in range(2):
                    c = slice(h * HS, (h + 1) * HS)
                    nc.vector.tensor_add(ps[:, c], ps[:, c],
                                         bias_t["o"][:, c])
                    nc.scalar.activation(so[:, c], ps[:, c], SIG)
                    # c_new = sig_f * (c0 + sig_i)
                    nc.vector.tensor_mul(cn[:, c], sig_f[bt][:, c],
                                         t1s[bt][:, c])
                    nc.sync.dma_start(out=co_r[bt][:, c], in_=cn[:, c])
                    # h_new = sig_o * tanh(c_new)
                    nc.scalar.activation(th[:, c], cn[:, c], TANH)
                    nc.vector.tensor_mul(hn[:, c], so[:, c], th[:, c])
                    nc.sync.dma_start(out=ho_r[bt][:, c], in_=hn[:, c])

    nc.compile()
    return nc


def _get_nc():
    if "nc" not in _CACHE:
        _CACHE["nc"] = _build_nc()
    return _CACHE["nc"]


def _prep_in_maps(inputs, h0, c0, ws, us, bs):
    """ws/us/bs: dicts g -> full array."""
    in_maps = []
    xts = []
    for blk in range(BB):
        rows = slice(blk * B_CORE, (blk + 1) * B_CORE)
        x = np.concatenate([inputs[rows], h0[rows]], axis=1)  # [1024, 2048]
        xts.append(np.ascontiguousarray(x.T).astype(np.float16))                 # [2048, 1024]
    wgs = {}
    for g in "fio":
        for sb in range(SB):
            cols = slice(sb * S_CORE, (sb + 1) * S_CORE)
            wgs[(g, sb)] = np.ascontiguousarray(
                np.concatenate([ws[g][:, cols], us[g][:, cols]],
                               axis=0)).astype(np.float16)
    for core in range(N_CORES):
        blk, sb = core // SB, core % SB
        rows = slice(blk * B_CORE, (blk + 1) * B_CORE)
        cols = slice(sb * S_CORE, (sb + 1) * S_CORE)
        m = {"xt": xts[blk], "c0": np.ascontiguousarray(c0[rows, cols])}
        for g in "fio":
            m[f"w{g}"] = wgs[(g, sb)]
            m[f"b{g}"] = np.ascontiguousarray(
                np.broadcast_to(bs[g][cols], (128, S_CORE)))
        in_maps.append(m)
    return in_maps


def _run(in_maps, trace=False, trace_kwargs=None, tmpdir=None):
    from concourse.bass_utils import run_bass_kernel_spmd

    nc = _get_nc()
    return run_bass_kernel_spmd(
        nc, in_maps, list(range(N_CORES)), trace=trace,
        trace_kwargs=trace_kwargs or {}, tmpdir=tmpdir,
    )


def _assemble(results):
    h = np.empty((B, S), dtype=np.float32)
    c = np.empty((B, S), dtype=np.float32)
    for core in range(N_CORES):
        blk, sb = core // SB, core % SB
        rows = slice(blk * B_CORE, (blk + 1) * B_CORE)
        cols = slice(sb * S_CORE, (sb + 1) * S_CORE)
        h[rows, cols] = results[core]["ho"]
        c[rows, cols] = results[core]["co"]
    return h, c


def kernel(inputs, h0, c0, w_f, u_f, b_f, w_i, u_i, b_i, w_o, u_o, b_o):
    inputs = np.asarray(inputs, dtype=np.float32)
    h0 = np.asarray(h0, dtype=np.float32)
    c0 = np.asarray(c0, dtype=np.float32)
    ws = {"f": np.asarray(w_f, np.float32), "i": np.asarray(w_i, np.float32),
          "o": np.asarray(w_o, np.float32)}
    us = {"f": np.asarray(u_f, np.float32), "i": np.asarray(u_i, np.float32),
          "o": np.asarray(u_o, np.float32)}
    bs = {"f": np.asarray(b_f, np.float32), "i": np.asarray(b_i, np.float32),
          "o": np.asarray(b_o, np.float32)}
    in_maps = _prep_in_maps(inputs, h0, c0, ws, us, bs)
    res = _run(in_maps)
    return _assemble(res.results)
